# revision 3
# baseline (speedup 1.0000x reference)
"""GQA kernel for Trainium2 (Bass/Tile), 8 NeuronCores.

Sharding: core c -> batch b=c//4, kv-head pair j=c%4 (kv heads 2j,2j+1,
q heads 8j..8j+7).  Each core computes out[b, :, 512j:512(j+1)].

Host pre-work: inputs are cast to bf16 and pre-transposed so the device
never transposes x or W:
  xT   [D, T]   = x[b].T
  wqT  [D, 512] = Wq rows (permuted: per QT tile m: head m evens, head m
                  odds, head m+4 evens, head m+4 odds).T
  wkT  [D, 128] = Wk rows (kv0 evens, kv0 odds, kv1 evens, kv1 odds).T
  wvT  [D, 128] = Wv rows (natural).T
  cosr [128, T] = cos.T tiled x4 (rows r use freq r%32)
  sins [128, T] = sin.T tiled x4 with sign -,+,-,+ per 32-row block

Device pipeline per core:
  1. Projections (PE, bf16): QT tiles [128,T] (q-dims on partitions),
     KT [128,T], V natural [T,128] with a ones column -> V' [128,130/t-tile].
  2. RoPE in transposed layout: out = raw*cosr + swap32(raw)*sins, where
     swap32 exchanges 32-row blocks (even<->odd dims) via SBUF->SBUF DMA.
  3. Attention per head h (kv=h//4, qt tile m=h%4), S^T layout
     (k on partitions, q on free):
       S^T band ki = matmul(lhsT=KT[64 rows, k-tile], rhs=QT_h[64, q>=k0])
       P^T = exp(S^T/8) (ACT, PSUM->SBUF bf16), no max subtraction
       diag 128x128 masked multiplicatively (gpsimd, upper-tri ones)
       O[qi] = sum_ki P^T(ki,qi-chunk).T @ [V|1]  -> [128, 65] PSUM
       col 64 accumulates the softmax denominator (ones column trick).
  4. Normalize: reciprocal(sums) (DVE), per-head tensor_scalar multiply,
     DMA out fp32.
"""

import sys

for _p in ("/opt/trn_rl_repo",):
    if _p not in sys.path:
        sys.path.insert(0, _p)

import numpy as np
import ml_dtypes

import concourse.bass as bass
import concourse.tile as tile
from concourse import bacc, mybir
from concourse.bass_utils import run_bass_kernel_spmd
from concourse.masks import make_upper_triangular

BF16 = mybir.dt.bfloat16
F32 = mybir.dt.float32
AF = mybir.ActivationFunctionType
ALU = mybir.AluOpType

D = 2048
HS = 64
SCALE = 1.0 / 8.0  # 1/sqrt(HS)


def _emit_body(tc, aps, T):
    nc = tc.nc
    NT = T // 128            # t tiles
    TCW = min(512, T)        # proj t-chunk width
    NTC = T // TCW
    ND = D // 128            # 16 contraction chunks
    SBW = min(1536, T)       # S^T sub-band width (3 PSUM banks)

    xT, wqT, wkT, wvT, cosr, sins, out = aps

    import contextlib
    ctx = tc._kernel_exitstack = contextlib.ExitStack()

    pers = ctx.enter_context(tc.tile_pool(name="pers", bufs=1))
    rp = ctx.enter_context(tc.tile_pool(name="rope", bufs=2))

    # ---- input loads ----
    xTs = []
    for di in range(ND):
        t = pers.tile([128, T], BF16, tag=f"xT{di}", name=f"xT{di}")
        nc.sync.dma_start(out=t[:], in_=xT[di * 128:(di + 1) * 128, :])
        xTs.append(t)
    wqTs, wkTs, wvTs = [], [], []
    for di in range(ND):
        t = pers.tile([128, 512], BF16, tag=f"wq{di}", name=f"wq{di}")
        nc.sync.dma_start(out=t[:], in_=wqT[di * 128:(di + 1) * 128, :])
        wqTs.append(t)
        t = pers.tile([128, 128], BF16, tag=f"wk{di}", name=f"wk{di}")
        nc.sync.dma_start(out=t[:], in_=wkT[di * 128:(di + 1) * 128, :])
        wkTs.append(t)
        t = pers.tile([128, 128], BF16, tag=f"wv{di}", name=f"wv{di}")
        nc.sync.dma_start(out=t[:], in_=wvT[di * 128:(di + 1) * 128, :])
        wvTs.append(t)
    cosr_t = pers.tile([128, T], BF16, tag="cosr")
    nc.sync.dma_start(out=cosr_t[:], in_=cosr[:, :])
    sins_t = pers.tile([128, T], BF16, tag="sins")
    nc.sync.dma_start(out=sins_t[:], in_=sins[:, :])

    triu = pers.tile([128, 128], BF16, tag="triu")
    make_upper_triangular(nc, triu[:], val=1.0, diag=True)

    # V' tiles: [kv0 64 | one | kv1 64 | one]
    vts = []
    for ti in range(NT):
        v = pers.tile([128, 130], BF16, tag=f"v{ti}", name=f"v{ti}")
        nc.vector.memset(v[:, 64:65], 1.0)
        nc.vector.memset(v[:, 129:130], 1.0)
        vts.append(v)

    qts = [pers.tile([128, T], BF16, tag=f"qt{m}", name=f"qt{m}") for m in range(4)]
    kt = pers.tile([128, T], BF16, tag="kt")

    osts = [pers.tile([128, 512], F32, tag=f"ost{qi}", name=f"ost{qi}") for qi in range(NT)]
    sums = pers.tile([128, NT * 8], F32, tag="sums")

    pts = [pers.tile([128, T - 128 * ki], BF16, tag=f"pt{ki}", name=f"pt{ki}") for ki in range(NT)]

    def rope(tgt):
        swp = rp.tile([128, T], BF16, tag="swp", name="swp")
        for (a, b) in ((0, 32), (32, 0), (64, 96), (96, 64)):
            nc.sync.dma_start(out=swp[a:a + 32, :], in_=tgt[b:b + 32, :])
        tmp = rp.tile([128, T], BF16, tag="tmp", name="tmp")
        nc.vector.tensor_tensor(out=tmp[:], in0=tgt[:], in1=cosr_t[:], op=ALU.mult)
        nc.vector.tensor_tensor(out=swp[:], in0=swp[:], in1=sins_t[:], op=ALU.mult)
        nc.vector.tensor_tensor(out=tgt[:], in0=tmp[:], in1=swp[:], op=ALU.add)

    # ---- projections ----
    with tc.tile_pool(name="pp", bufs=6, space="PSUM") as pp:
        # K: out KT chunk [128 kdim, TCW]
        for tcI in range(NTC):
            ps = pp.tile([128, TCW], F32, tag="pj", name="pj")
            for di in range(ND):
                nc.tensor.matmul(
                    ps[:], wkTs[di][:], xTs[di][:, tcI * TCW:(tcI + 1) * TCW],
                    start=(di == 0), stop=(di == ND - 1))
            nc.scalar.copy(kt[:, tcI * TCW:(tcI + 1) * TCW], ps[:])
        # Q tile m=0
        for tcI in range(NTC):
            ps = pp.tile([128, TCW], F32, tag="pj", name="pj")
            for di in range(ND):
                nc.tensor.matmul(
                    ps[:], wqTs[di][:, 0:128], xTs[di][:, tcI * TCW:(tcI + 1) * TCW],
                    start=(di == 0), stop=(di == ND - 1))
            nc.scalar.copy(qts[0][:, tcI * TCW:(tcI + 1) * TCW], ps[:])
        rope(kt)
        rope(qts[0])
        # V: natural layout; one psum tile per t-tile
        for ti in range(NT):
            ps = pp.tile([128, 128], F32, tag="pj", name="pjv")
            for di in range(ND):
                nc.tensor.matmul(
                    ps[:], xTs[di][:, ti * 128:(ti + 1) * 128], wvTs[di][:],
                    start=(di == 0), stop=(di == ND - 1))
            nc.scalar.copy(vts[ti][:, 0:64], ps[:, 0:64])
            nc.scalar.copy(vts[ti][:, 65:129], ps[:, 64:128])
        # Q tiles m=1..3
        for m in range(1, 4):
            for tcI in range(NTC):
                ps = pp.tile([128, TCW], F32, tag="pj", name="pj")
                for di in range(ND):
                    nc.tensor.matmul(
                        ps[:], wqTs[di][:, m * 128:(m + 1) * 128],
                        xTs[di][:, tcI * TCW:(tcI + 1) * TCW],
                        start=(di == 0), stop=(di == ND - 1))
                nc.scalar.copy(qts[m][:, tcI * TCW:(tcI + 1) * TCW], ps[:])
            rope(qts[m])

    # ---- attention ----
    sp = ctx.enter_context(tc.tile_pool(name="spsum", bufs=2, space="PSUM"))
    op_ = ctx.enter_context(tc.tile_pool(name="opsum", bufs=2, space="PSUM"))

    head_order = [0, 4, 1, 5, 2, 6, 3, 7]
    for h in head_order:
        m = h % 4
        base = (h // 4) * 64
        kv = h // 4
        for n in range(NT):          # band index = k-tile = n; also qi = n
            ki = n
            w = T - 128 * ki
            q0 = 128 * ki
            # S^T band in sub-bands of <=SBW
            for sb in range(0, w, SBW):
                sw = min(SBW, w - sb)
                s_ps = sp.tile([128, SBW], F32, tag="s", name="s")
                for off in range(0, sw, 512):
                    cn = min(512, sw - off)
                    nc.tensor.matmul(
                        s_ps[:, off:off + cn],
                        kt[base:base + 64, ki * 128:(ki + 1) * 128],
                        qts[m][base:base + 64, q0 + sb + off:q0 + sb + off + cn],
                        start=True, stop=True)
                nc.scalar.activation(pts[ki][:, sb:sb + sw], s_ps[:, 0:sw],
                                     AF.Exp, scale=SCALE)
            nc.gpsimd.tensor_tensor(out=pts[ki][:, 0:128], in0=pts[ki][:, 0:128],
                                    in1=triu[:], op=ALU.mult)
            # PV for q-tile qi=n over all ki' <= n
            o_ps = op_.tile([128, 65], F32, tag="o", name="o")
            for kp in range(n + 1):
                nc.tensor.matmul(
                    o_ps[:],
                    pts[kp][:, (n - kp) * 128:(n - kp + 1) * 128],
                    vts[kp][:, kv * 65:kv * 65 + 65],
                    start=(kp == 0), stop=(kp == n))
            nc.vector.tensor_copy(osts[n][:, h * 64:(h + 1) * 64], o_ps[:, 0:64])
            nc.vector.tensor_copy(sums[:, n * 8 + h:n * 8 + h + 1], o_ps[:, 64:65])

    # ---- normalize + store ----
    recs = pers.tile([128, NT * 8], F32, tag="recs")
    nc.vector.reciprocal(recs[:], sums[:])
    for qi in range(NT):
        for h in range(8):
            nc.vector.tensor_scalar_mul(
                osts[qi][:, h * 64:(h + 1) * 64],
                osts[qi][:, h * 64:(h + 1) * 64],
                recs[:, qi * 8 + h:qi * 8 + h + 1])
        nc.sync.dma_start(out=out[qi * 128:(qi + 1) * 128, :], in_=osts[qi][:])

    ctx.close()


def build_program(T=2048, num_devices=8):
    nc = bacc.Bacc("TRN2", target_bir_lowering=False, debug=False,
                   num_devices=num_devices)
    xT = nc.dram_tensor("xT", (D, T), BF16, kind="ExternalInput").ap()
    wqT = nc.dram_tensor("wqT", (D, 512), BF16, kind="ExternalInput").ap()
    wkT = nc.dram_tensor("wkT", (D, 128), BF16, kind="ExternalInput").ap()
    wvT = nc.dram_tensor("wvT", (D, 128), BF16, kind="ExternalInput").ap()
    cosr = nc.dram_tensor("cosr", (128, T), BF16, kind="ExternalInput").ap()
    sins = nc.dram_tensor("sins", (128, T), BF16, kind="ExternalInput").ap()
    out = nc.dram_tensor("out", (T, 512), F32, kind="ExternalOutput").ap()
    with tile.TileContext(nc) as tc:
        _emit_body(tc, (xT, wqT, wkT, wvT, cosr, sins, out), T)
    nc.compile()
    return nc


# ---------------- host side ----------------

def _qperm(j):
    rows = []
    for m in range(4):
        for r in range(128):
            h = m if r < 64 else m + 4
            d = 2 * (r % 32) + (1 if (r % 64) >= 32 else 0)
            rows.append((8 * j + h) * 64 + d)
    return np.array(rows)


def _kperm(j):
    rows = []
    for kv in range(2):
        for r in range(64):
            d = 2 * (r % 32) + (1 if r >= 32 else 0)
            rows.append((2 * j + kv) * 64 + d)
    return np.array(rows)


def make_core_inputs(x, Wq, Wk, Wv, cos, sin):
    """Per-core input dicts (host prep). x: [B,T,D]."""
    bf = ml_dtypes.bfloat16
    B, T, _ = x.shape
    xTb = [np.ascontiguousarray(x[b].T).astype(bf) for b in range(B)]
    cosT = np.ascontiguousarray(cos.T.astype(np.float32))  # [32, T]
    sinT = np.ascontiguousarray(sin.T.astype(np.float32))
    cosr = np.tile(cosT, (4, 1)).astype(bf)
    sgn = np.repeat(np.array([-1.0, 1.0, -1.0, 1.0], np.float32), 32)
    sins = (np.tile(sinT, (4, 1)) * sgn[:, None]).astype(bf)
    maps = []
    for c in range(8):
        b, j = c // 4, c % 4
        maps.append({
            "xT": xTb[b],
            "wqT": np.ascontiguousarray(Wq[_qperm(j)].T).astype(bf),
            "wkT": np.ascontiguousarray(Wk[_kperm(j)].T).astype(bf),
            "wvT": np.ascontiguousarray(Wv[128 * j:128 * (j + 1)].T).astype(bf),
            "cosr": cosr,
            "sins": sins,
        })
    return maps


_CACHE = {}


def _get_program():
    if "nc" not in _CACHE:
        _CACHE["nc"] = build_program(T=2048, num_devices=8)
    return _CACHE["nc"]


def run_on_hw(in_maps, trace=False):
    nc = _get_program()
    return run_bass_kernel_spmd(nc, in_maps, list(range(8)), trace=trace)


def kernel(x, Wq, Wk, Wv, cos, sin):
    x = np.asarray(x, np.float32)
    Wq = np.asarray(Wq, np.float32)
    Wk = np.asarray(Wk, np.float32)
    Wv = np.asarray(Wv, np.float32)
    cos = np.asarray(cos, np.float32)
    sin = np.asarray(sin, np.float32)
    maps = make_core_inputs(x, Wq, Wk, Wv, cos, sin)
    res = run_on_hw(maps, trace=False)
    B, T = x.shape[0], x.shape[1]
    out = np.empty((B, T, 2048), np.float32)
    for c in range(8):
        b, j = c // 4, c % 4
        out[b, :, 512 * j:512 * (j + 1)] = res.results[c]["out"]
    return out


# revision 5
# speedup vs baseline: 1.1306x; 1.1306x over previous
"""GQA kernel for Trainium2 (Bass/Tile), 8 NeuronCores.

Sharding: core c -> batch b=c//4, kv-head pair j=c%4 (kv heads 2j,2j+1,
q heads 8j..8j+7).  Each core computes out[b, :, 512j:512(j+1)].

Host pre-work: inputs are cast to bf16 and pre-transposed so the device
never transposes x or W:
  xT   [D, T]   = x[b].T
  wqT  [D, 512] = Wq rows (permuted: per QT tile m: head m evens, head m
                  odds, head m+4 evens, head m+4 odds).T
  wkT  [D, 128] = Wk rows (kv0 evens, kv0 odds, kv1 evens, kv1 odds).T
  wvT  [D, 128] = Wv rows (natural).T
  cosr [128, T] = cos.T tiled x4 (rows r use freq r%32)
  sins [128, T] = sin.T tiled x4 with sign -,+,-,+ per 32-row block

Device pipeline per core:
  1. Projections (PE, bf16): QT tiles [128,T] (q-dims on partitions),
     KT [128,T], V natural [T,128] with a ones column -> V' [128,130/t-tile].
  2. RoPE in transposed layout: out = raw*cosr + swap32(raw)*sins, where
     swap32 exchanges 32-row blocks (even<->odd dims) via SBUF->SBUF DMA.
  3. Attention per head h (kv=h//4, qt tile m=h%4), S^T layout
     (k on partitions, q on free):
       S^T band ki = matmul(lhsT=KT[64 rows, k-tile], rhs=QT_h[64, q>=k0])
       P^T = exp(S^T/8) (ACT, PSUM->SBUF bf16), no max subtraction
       diag 128x128 masked multiplicatively (gpsimd, upper-tri ones)
       O[qi] = sum_ki P^T(ki,qi-chunk).T @ [V|1]  -> [128, 65] PSUM
       col 64 accumulates the softmax denominator (ones column trick).
  4. Normalize: reciprocal(sums) (DVE), per-head tensor_scalar multiply,
     DMA out fp32.
"""

import sys

for _p in ("/opt/trn_rl_repo",):
    if _p not in sys.path:
        sys.path.insert(0, _p)

import numpy as np
import ml_dtypes

import concourse.bass as bass
import concourse.tile as tile
from concourse import bacc, mybir
from concourse.bass_utils import run_bass_kernel_spmd
from concourse.masks import make_upper_triangular

BF16 = mybir.dt.bfloat16
F32 = mybir.dt.float32
AF = mybir.ActivationFunctionType
ALU = mybir.AluOpType

D = 2048
HS = 64
SCALE = 1.0 / 8.0  # 1/sqrt(HS)


def _emit_body(tc, aps, T):
    nc = tc.nc
    NT = T // 128            # t tiles
    TCW = min(512, T)        # proj t-chunk width
    NTC = T // TCW
    ND = D // 128            # 16 contraction chunks
    SBW = min(1536, T)       # S^T sub-band width (3 PSUM banks)

    xT, wqT, wkT, wvT, cosr, sins, out = aps

    import contextlib
    ctx = tc._kernel_exitstack = contextlib.ExitStack()

    pers = ctx.enter_context(tc.tile_pool(name="pers", bufs=1))
    rp = ctx.enter_context(tc.tile_pool(name="rope", bufs=2))

    # ---- input loads (interleaved so di=0 tiles of every tensor land first)
    xTs, wqTs, wkTs, wvTs = [], [], [], []
    for di in range(ND):
        t = pers.tile([128, 128], BF16, tag=f"wk{di}", name=f"wk{di}")
        nc.sync.dma_start(out=t[:], in_=wkT[di * 128:(di + 1) * 128, :])
        wkTs.append(t)
        t = pers.tile([128, 512], BF16, tag=f"wq{di}", name=f"wq{di}")
        nc.sync.dma_start(out=t[:], in_=wqT[di * 128:(di + 1) * 128, :])
        wqTs.append(t)
        t = pers.tile([128, 128], BF16, tag=f"wv{di}", name=f"wv{di}")
        nc.sync.dma_start(out=t[:], in_=wvT[di * 128:(di + 1) * 128, :])
        wvTs.append(t)
        t = pers.tile([128, T], BF16, tag=f"xT{di}", name=f"xT{di}")
        nc.sync.dma_start(out=t[:], in_=xT[di * 128:(di + 1) * 128, :])
        xTs.append(t)
    cosr_t = pers.tile([128, T], BF16, tag="cosr")
    nc.sync.dma_start(out=cosr_t[:], in_=cosr[:, :])
    sins_t = pers.tile([128, T], BF16, tag="sins")
    nc.sync.dma_start(out=sins_t[:], in_=sins[:, :])

    triu = pers.tile([128, 128], BF16, tag="triu")
    make_upper_triangular(nc, triu[:], val=1.0, diag=True)

    # V' tiles: [kv0 64 | one | kv1 64 | one]
    vts = []
    for ti in range(NT):
        v = pers.tile([128, 130], BF16, tag=f"v{ti}", name=f"v{ti}")
        nc.vector.memset(v[:, 64:65], 1.0)
        nc.vector.memset(v[:, 129:130], 1.0)
        vts.append(v)

    qts = [pers.tile([128, T], BF16, tag=f"qt{m}", name=f"qt{m}") for m in range(4)]
    kt = pers.tile([128, T], BF16, tag="kt")

    osts = [pers.tile([128, 512], F32, tag=f"ost{qi}", name=f"ost{qi}") for qi in range(NT)]
    sums = pers.tile([128, NT * 8], F32, tag="sums")

    pts = [pers.tile([128, T - 128 * ki], BF16, tag=f"pt{ki}", name=f"pt{ki}") for ki in range(NT)]

    def rope(tgt):
        swp = rp.tile([128, T], BF16, tag="swp", name="swp")
        for (a, b) in ((0, 32), (32, 0), (64, 96), (96, 64)):
            nc.sync.dma_start(out=swp[a:a + 32, :], in_=tgt[b:b + 32, :])
        tmp = rp.tile([128, T], BF16, tag="tmp", name="tmp")
        nc.vector.tensor_tensor(out=tmp[:], in0=tgt[:], in1=cosr_t[:], op=ALU.mult)
        nc.vector.tensor_tensor(out=swp[:], in0=swp[:], in1=sins_t[:], op=ALU.mult)
        nc.vector.tensor_tensor(out=tgt[:], in0=tmp[:], in1=swp[:], op=ALU.add)

    # ---- projection group helpers ----
    def q_group(pool, m, tcI):
        ps = pool.tile([128, TCW], F32, tag="pj", name="pj")
        for di in range(ND):
            nc.tensor.matmul(
                ps[:], wqTs[di][:, m * 128:(m + 1) * 128],
                xTs[di][:, tcI * TCW:(tcI + 1) * TCW],
                start=(di == 0), stop=(di == ND - 1))
        nc.vector.tensor_copy(qts[m][:, tcI * TCW:(tcI + 1) * TCW], ps[:])

    def k_group(pool, tcI):
        ps = pool.tile([128, TCW], F32, tag="pj", name="pj")
        for di in range(ND):
            nc.tensor.matmul(
                ps[:], wkTs[di][:], xTs[di][:, tcI * TCW:(tcI + 1) * TCW],
                start=(di == 0), stop=(di == ND - 1))
        nc.vector.tensor_copy(kt[:, tcI * TCW:(tcI + 1) * TCW], ps[:])

    def v_group(pool, ti):
        ps = pool.tile([128, 128], F32, tag="pj", name="pjv")
        for di in range(ND):
            nc.tensor.matmul(
                ps[:], xTs[di][:, ti * 128:(ti + 1) * 128], wvTs[di][:],
                start=(di == 0), stop=(di == ND - 1))
        nc.vector.tensor_copy(vts[ti][:, 0:64], ps[:, 0:64])
        nc.vector.tensor_copy(vts[ti][:, 65:129], ps[:, 64:128])

    # ---- pre-attention projections: K, Q0, V (4-bank pool, scoped) ----
    with tc.tile_pool(name="ppe", bufs=4, space="PSUM") as ppe:
        for tcI in range(NTC):
            k_group(ppe, tcI)
        for tcI in range(NTC):
            q_group(ppe, 0, tcI)
        rope(kt)
        rope(qts[0])
        for ti in range(NT):
            v_group(ppe, ti)

    # ---- attention with woven projection groups (PE warmth) ----
    sp = ctx.enter_context(tc.tile_pool(name="spsum", bufs=2, space="PSUM"))
    op_ = ctx.enter_context(tc.tile_pool(name="opsum", bufs=1, space="PSUM"))
    ppl = ctx.enter_context(tc.tile_pool(name="ppl", bufs=1, space="PSUM"))

    def attn_head(h, fillers=(), fill_bands=(3, 6, 9, 12)):
        m = h % 4
        base = (h // 4) * 64
        kv = h // 4
        fills = list(fillers)
        for n in range(NT):
            ki = n
            w = T - 128 * ki
            q0 = 128 * ki
            for sb in range(0, w, SBW):
                sw = min(SBW, w - sb)
                s_ps = sp.tile([128, SBW], F32, tag="s", name="s")
                for off in range(0, sw, 512):
                    cn = min(512, sw - off)
                    nc.tensor.matmul(
                        s_ps[:, off:off + cn],
                        kt[base:base + 64, ki * 128:(ki + 1) * 128],
                        qts[m][base:base + 64, q0 + sb + off:q0 + sb + off + cn],
                        start=True, stop=True)
                nc.scalar.activation(pts[ki][:, sb:sb + sw], s_ps[:, 0:sw],
                                     AF.Exp, scale=SCALE)
            nc.gpsimd.tensor_tensor(out=pts[ki][:, 0:128], in0=pts[ki][:, 0:128],
                                    in1=triu[:], op=ALU.mult)
            o_ps = op_.tile([128, 65], F32, tag="o", name="o")
            for kp in range(n + 1):
                nc.tensor.matmul(
                    o_ps[:],
                    pts[kp][:, (n - kp) * 128:(n - kp + 1) * 128],
                    vts[kp][:, kv * 65:kv * 65 + 65],
                    start=(kp == 0), stop=(kp == n))
            nc.vector.tensor_copy(osts[n][:, h * 64:(h + 1) * 64], o_ps[:, 0:64])
            nc.vector.tensor_copy(sums[:, n * 8 + h:n * 8 + h + 1], o_ps[:, 64:65])
            if n in fill_bands and fills:
                fills.pop(0)()
        assert not fills, "unemitted filler projection groups"

    # head 0 woven with Q1 proj; head 4 with Q2; head 1 with Q3
    attn_head(0, [lambda tcI=t: q_group(ppl, 1, tcI) for t in range(NTC)])
    rope(qts[1])
    attn_head(4, [lambda tcI=t: q_group(ppl, 2, tcI) for t in range(NTC)])
    rope(qts[2])
    attn_head(1, [lambda tcI=t: q_group(ppl, 3, tcI) for t in range(NTC)])
    rope(qts[3])
    for h in (5, 2, 6, 3, 7):
        attn_head(h)

    # ---- normalize + store ----
    recs = pers.tile([128, NT * 8], F32, tag="recs")
    nc.vector.reciprocal(recs[:], sums[:])
    for qi in range(NT):
        for h in range(8):
            nc.vector.tensor_scalar_mul(
                osts[qi][:, h * 64:(h + 1) * 64],
                osts[qi][:, h * 64:(h + 1) * 64],
                recs[:, qi * 8 + h:qi * 8 + h + 1])
        nc.sync.dma_start(out=out[qi * 128:(qi + 1) * 128, :], in_=osts[qi][:])

    ctx.close()


def build_program(T=2048, num_devices=8):
    nc = bacc.Bacc("TRN2", target_bir_lowering=False, debug=False,
                   num_devices=num_devices)
    xT = nc.dram_tensor("xT", (D, T), BF16, kind="ExternalInput").ap()
    wqT = nc.dram_tensor("wqT", (D, 512), BF16, kind="ExternalInput").ap()
    wkT = nc.dram_tensor("wkT", (D, 128), BF16, kind="ExternalInput").ap()
    wvT = nc.dram_tensor("wvT", (D, 128), BF16, kind="ExternalInput").ap()
    cosr = nc.dram_tensor("cosr", (128, T), BF16, kind="ExternalInput").ap()
    sins = nc.dram_tensor("sins", (128, T), BF16, kind="ExternalInput").ap()
    out = nc.dram_tensor("out", (T, 512), F32, kind="ExternalOutput").ap()
    with tile.TileContext(nc) as tc:
        _emit_body(tc, (xT, wqT, wkT, wvT, cosr, sins, out), T)
    nc.compile()
    return nc


# ---------------- host side ----------------

def _qperm(j):
    rows = []
    for m in range(4):
        for r in range(128):
            h = m if r < 64 else m + 4
            d = 2 * (r % 32) + (1 if (r % 64) >= 32 else 0)
            rows.append((8 * j + h) * 64 + d)
    return np.array(rows)


def _kperm(j):
    rows = []
    for kv in range(2):
        for r in range(64):
            d = 2 * (r % 32) + (1 if r >= 32 else 0)
            rows.append((2 * j + kv) * 64 + d)
    return np.array(rows)


def make_core_inputs(x, Wq, Wk, Wv, cos, sin):
    """Per-core input dicts (host prep). x: [B,T,D]."""
    bf = ml_dtypes.bfloat16
    B, T, _ = x.shape
    xTb = [np.ascontiguousarray(x[b].T).astype(bf) for b in range(B)]
    cosT = np.ascontiguousarray(cos.T.astype(np.float32))  # [32, T]
    sinT = np.ascontiguousarray(sin.T.astype(np.float32))
    cosr = np.tile(cosT, (4, 1)).astype(bf)
    sgn = np.repeat(np.array([-1.0, 1.0, -1.0, 1.0], np.float32), 32)
    sins = (np.tile(sinT, (4, 1)) * sgn[:, None]).astype(bf)
    maps = []
    for c in range(8):
        b, j = c // 4, c % 4
        maps.append({
            "xT": xTb[b],
            "wqT": np.ascontiguousarray(Wq[_qperm(j)].T).astype(bf),
            "wkT": np.ascontiguousarray(Wk[_kperm(j)].T).astype(bf),
            "wvT": np.ascontiguousarray(Wv[128 * j:128 * (j + 1)].T).astype(bf),
            "cosr": cosr,
            "sins": sins,
        })
    return maps


_CACHE = {}


def _get_program():
    if "nc" not in _CACHE:
        _CACHE["nc"] = build_program(T=2048, num_devices=8)
    return _CACHE["nc"]


def run_on_hw(in_maps, trace=False):
    nc = _get_program()
    return run_bass_kernel_spmd(nc, in_maps, list(range(8)), trace=trace)


def kernel(x, Wq, Wk, Wv, cos, sin):
    x = np.asarray(x, np.float32)
    Wq = np.asarray(Wq, np.float32)
    Wk = np.asarray(Wk, np.float32)
    Wv = np.asarray(Wv, np.float32)
    cos = np.asarray(cos, np.float32)
    sin = np.asarray(sin, np.float32)
    maps = make_core_inputs(x, Wq, Wk, Wv, cos, sin)
    res = run_on_hw(maps, trace=False)
    B, T = x.shape[0], x.shape[1]
    out = np.empty((B, T, 2048), np.float32)
    for c in range(8):
        b, j = c // 4, c % 4
        out[b, :, 512 * j:512 * (j + 1)] = res.results[c]["out"]
    return out


# revision 7
# speedup vs baseline: 1.1409x; 1.0091x over previous
"""GQA kernel for Trainium2 (Bass/Tile), 8 NeuronCores.

Sharding: core c -> batch b=c//4, kv-head pair j=c%4 (kv heads 2j,2j+1,
q heads 8j..8j+7).  Each core computes out[b, :, 512j:512(j+1)].

Host pre-work: inputs are cast to bf16 and pre-transposed so the device
never transposes x or W:
  xT   [D, T]   = x[b].T
  wqT  [D, 512] = Wq rows (permuted: per QT tile m: head m evens, head m
                  odds, head m+4 evens, head m+4 odds).T
  wkT  [D, 128] = Wk rows (kv0 evens, kv0 odds, kv1 evens, kv1 odds).T
  wvT  [D, 128] = Wv rows (natural).T
  cosr [128, T] = cos.T tiled x4 (rows r use freq r%32)
  sins [128, T] = sin.T tiled x4 with sign -,+,-,+ per 32-row block

Device pipeline per core:
  1. Projections (PE, bf16): QT tiles [128,T] (q-dims on partitions),
     KT [128,T], V natural [T,128] with a ones column -> V' [128,130/t-tile].
  2. RoPE in transposed layout: out = raw*cosr + swap32(raw)*sins, where
     swap32 exchanges 32-row blocks (even<->odd dims) via SBUF->SBUF DMA.
  3. Attention per head h (kv=h//4, qt tile m=h%4), S^T layout
     (k on partitions, q on free):
       S^T band ki = matmul(lhsT=KT[64 rows, k-tile], rhs=QT_h[64, q>=k0])
       P^T = exp(S^T/8) (ACT, PSUM->SBUF bf16), no max subtraction
       diag 128x128 masked multiplicatively (gpsimd, upper-tri ones)
       O[qi] = sum_ki P^T(ki,qi-chunk).T @ [V|1]  -> [128, 65] PSUM
       col 64 accumulates the softmax denominator (ones column trick).
  4. Normalize: reciprocal(sums) (DVE), per-head tensor_scalar multiply,
     DMA out fp32.
"""

import sys

for _p in ("/opt/trn_rl_repo",):
    if _p not in sys.path:
        sys.path.insert(0, _p)

import numpy as np
import ml_dtypes

import concourse.bass as bass
import concourse.tile as tile
from concourse import bacc, mybir
from concourse.bass_utils import run_bass_kernel_spmd
from concourse.masks import make_upper_triangular

BF16 = mybir.dt.bfloat16
F32 = mybir.dt.float32
AF = mybir.ActivationFunctionType
ALU = mybir.AluOpType

D = 2048
HS = 64
SCALE = 1.0 / 8.0  # 1/sqrt(HS)


def _emit_body(tc, aps, T):
    nc = tc.nc
    NT = T // 128            # t tiles
    TCW = min(512, T)        # proj t-chunk width
    NTC = T // TCW
    ND = D // 128            # 16 contraction chunks
    SBW = min(1536, T)       # S^T sub-band width (3 PSUM banks)

    xT, wqT, wkT, wvT, cosr, sins, out = aps

    import contextlib
    ctx = tc._kernel_exitstack = contextlib.ExitStack()

    pers = ctx.enter_context(tc.tile_pool(name="pers", bufs=1))
    rp = ctx.enter_context(tc.tile_pool(name="rope", bufs=2))

    # ---- input loads (interleaved so di=0 tiles of every tensor land first)
    xTs, wqTs, wkTs, wvTs = [], [], [], []
    for di in range(ND):
        t = pers.tile([128, 128], BF16, tag=f"wk{di}", name=f"wk{di}")
        nc.sync.dma_start(out=t[:], in_=wkT[di * 128:(di + 1) * 128, :])
        wkTs.append(t)
        t = pers.tile([128, 512], BF16, tag=f"wq{di}", name=f"wq{di}")
        nc.sync.dma_start(out=t[:], in_=wqT[di * 128:(di + 1) * 128, :])
        wqTs.append(t)
        t = pers.tile([128, 128], BF16, tag=f"wv{di}", name=f"wv{di}")
        nc.sync.dma_start(out=t[:], in_=wvT[di * 128:(di + 1) * 128, :])
        wvTs.append(t)
        t = pers.tile([128, T], BF16, tag=f"xT{di}", name=f"xT{di}")
        nc.sync.dma_start(out=t[:], in_=xT[di * 128:(di + 1) * 128, :])
        xTs.append(t)
    cosr_t = pers.tile([128, T], BF16, tag="cosr")
    nc.sync.dma_start(out=cosr_t[:], in_=cosr[:, :])
    sins_t = pers.tile([128, T], BF16, tag="sins")
    nc.sync.dma_start(out=sins_t[:], in_=sins[:, :])

    triu = pers.tile([128, 128], BF16, tag="triu")
    make_upper_triangular(nc, triu[:], val=1.0, diag=True)

    # V' tiles: [kv0 64 | one | kv1 64 | one]
    vts = []
    for ti in range(NT):
        v = pers.tile([128, 130], BF16, tag=f"v{ti}", name=f"v{ti}")
        nc.vector.memset(v[:, 64:65], 1.0)
        nc.vector.memset(v[:, 129:130], 1.0)
        vts.append(v)

    qts = [pers.tile([128, T], BF16, tag=f"qt{m}", name=f"qt{m}") for m in range(4)]
    kt = pers.tile([128, T], BF16, tag="kt")

    osts = [pers.tile([128, 512], F32, tag=f"ost{qi}", name=f"ost{qi}") for qi in range(NT)]
    sums = pers.tile([128, NT * 8], F32, tag="sums")

    pts = [pers.tile([128, T - 128 * ki], BF16, tag=f"pt{ki}", name=f"pt{ki}") for ki in range(NT)]

    def rope(tgt):
        swp = rp.tile([128, T], BF16, tag="swp", name="swp")
        for (a, b) in ((0, 32), (32, 0), (64, 96), (96, 64)):
            nc.sync.dma_start(out=swp[a:a + 32, :], in_=tgt[b:b + 32, :])
        tmp = rp.tile([128, T], BF16, tag="tmp", name="tmp")
        nc.vector.tensor_tensor(out=tmp[:], in0=tgt[:], in1=cosr_t[:], op=ALU.mult)
        nc.vector.tensor_tensor(out=swp[:], in0=swp[:], in1=sins_t[:], op=ALU.mult)
        nc.vector.tensor_tensor(out=tgt[:], in0=tmp[:], in1=swp[:], op=ALU.add)

    # ---- projection group helpers ----
    def q_group(pool, m, tcI):
        ps = pool.tile([128, TCW], F32, tag="pj", name="pj")
        for di in range(ND):
            nc.tensor.matmul(
                ps[:], wqTs[di][:, m * 128:(m + 1) * 128],
                xTs[di][:, tcI * TCW:(tcI + 1) * TCW],
                start=(di == 0), stop=(di == ND - 1))
        nc.vector.tensor_copy(qts[m][:, tcI * TCW:(tcI + 1) * TCW], ps[:])

    def k_group(pool, tcI):
        ps = pool.tile([128, TCW], F32, tag="pj", name="pj")
        for di in range(ND):
            nc.tensor.matmul(
                ps[:], wkTs[di][:], xTs[di][:, tcI * TCW:(tcI + 1) * TCW],
                start=(di == 0), stop=(di == ND - 1))
        nc.vector.tensor_copy(kt[:, tcI * TCW:(tcI + 1) * TCW], ps[:])

    def v_group(pool, ti):
        ps = pool.tile([128, 128], F32, tag="pj", name="pjv")
        for di in range(ND):
            nc.tensor.matmul(
                ps[:], xTs[di][:, ti * 128:(ti + 1) * 128], wvTs[di][:],
                start=(di == 0), stop=(di == ND - 1))
        nc.vector.tensor_copy(vts[ti][:, 0:64], ps[:, 0:64])
        nc.vector.tensor_copy(vts[ti][:, 65:129], ps[:, 64:128])

    # ---- pre-attention projections: K, Q0, V (4-bank pool, scoped) ----
    with tc.tile_pool(name="ppe", bufs=4, space="PSUM") as ppe:
        for tcI in range(NTC):
            k_group(ppe, tcI)
        for tcI in range(NTC):
            q_group(ppe, 0, tcI)
        rope(kt)
        rope(qts[0])
        for ti in range(NT):
            v_group(ppe, ti)

    # ---- attention with woven projection groups (PE warmth) ----
    sp = ctx.enter_context(tc.tile_pool(name="spsum", bufs=2, space="PSUM"))
    op_ = ctx.enter_context(tc.tile_pool(name="opsum", bufs=1, space="PSUM"))
    ppl = ctx.enter_context(tc.tile_pool(name="ppl", bufs=1, space="PSUM"))

    def dummy_burst(nmm=16):
        # back-to-back PE work to re-ignite the HAM clock gate (output unused)
        ps = ppl.tile([128, 512], F32, tag="pj", name="pjw")
        for i in range(nmm):
            nc.tensor.matmul(ps[:], kt[0:64, 0:128], qts[0][0:64, 0:512],
                             start=True, stop=True)

    def attn_head(h, fillers=(), fill_bands=(2, 5, 8, 11, 14)):
        m = h % 4
        base = (h // 4) * 64
        kv = h // 4
        fills = list(fillers)
        for n in range(NT):
            ki = n
            w = T - 128 * ki
            q0 = 128 * ki
            for sb in range(0, w, SBW):
                sw = min(SBW, w - sb)
                s_ps = sp.tile([128, SBW], F32, tag="s", name="s")
                for off in range(0, sw, 512):
                    cn = min(512, sw - off)
                    nc.tensor.matmul(
                        s_ps[:, off:off + cn],
                        kt[base:base + 64, ki * 128:(ki + 1) * 128],
                        qts[m][base:base + 64, q0 + sb + off:q0 + sb + off + cn],
                        start=True, stop=True)
                nc.scalar.activation(pts[ki][:, sb:sb + sw], s_ps[:, 0:sw],
                                     AF.Exp, scale=SCALE)
            nc.gpsimd.tensor_tensor(out=pts[ki][:, 0:128], in0=pts[ki][:, 0:128],
                                    in1=triu[:], op=ALU.mult)
            o_ps = op_.tile([128, 65], F32, tag="o", name="o")
            for kp in range(n + 1):
                nc.tensor.matmul(
                    o_ps[:],
                    pts[kp][:, (n - kp) * 128:(n - kp + 1) * 128],
                    vts[kp][:, kv * 65:kv * 65 + 65],
                    start=(kp == 0), stop=(kp == n))
            nc.vector.tensor_copy(osts[n][:, h * 64:(h + 1) * 64], o_ps[:, 0:64])
            nc.vector.tensor_copy(sums[:, n * 8 + h:n * 8 + h + 1], o_ps[:, 64:65])
            if n in fill_bands:
                if fills:
                    fills.pop(0)()
                elif n in (2, 8):
                    dummy_burst()
        assert not fills, "unemitted filler projection groups"

    # spread Q1/Q2/Q3 projection groups across early heads as PE warmers:
    # Q1 done before attn(1), Q2 before attn(2), Q3 before attn(3)
    g = [lambda m=mm, tcI=t: q_group(ppl, m, tcI)
         for mm in (1, 2, 3) for t in range(NTC)]
    nf = len(g) // 6 or 1
    if NTC == 4:
        plan = [(0, g[0:2]), (4, g[2:4]), (1, g[4:6]), (5, g[6:8]),
                (2, g[8:10]), (6, g[10:12]), (3, []), (7, [])]
        ropes = {4: 1, 5: 2, 6: 3}
    else:  # small-T sim: one filler per early head
        plan = [(0, g[0:1]), (4, g[1:2]), (1, g[2:3]), (5, []), (2, []),
                (6, []), (3, []), (7, [])]
        ropes = {0: 1, 4: 2, 1: 3}
    for h, fl in plan:
        attn_head(h, fl)
        if h in ropes:
            rope(qts[ropes[h]])

    # ---- normalize + store ----
    recs = pers.tile([128, NT * 8], F32, tag="recs")
    nc.vector.reciprocal(recs[:], sums[:])
    for qi in range(NT):
        for h in range(8):
            nc.vector.tensor_scalar_mul(
                osts[qi][:, h * 64:(h + 1) * 64],
                osts[qi][:, h * 64:(h + 1) * 64],
                recs[:, qi * 8 + h:qi * 8 + h + 1])
        nc.sync.dma_start(out=out[qi * 128:(qi + 1) * 128, :], in_=osts[qi][:])

    ctx.close()


def build_program(T=2048, num_devices=8):
    nc = bacc.Bacc("TRN2", target_bir_lowering=False, debug=False,
                   num_devices=num_devices)
    xT = nc.dram_tensor("xT", (D, T), BF16, kind="ExternalInput").ap()
    wqT = nc.dram_tensor("wqT", (D, 512), BF16, kind="ExternalInput").ap()
    wkT = nc.dram_tensor("wkT", (D, 128), BF16, kind="ExternalInput").ap()
    wvT = nc.dram_tensor("wvT", (D, 128), BF16, kind="ExternalInput").ap()
    cosr = nc.dram_tensor("cosr", (128, T), BF16, kind="ExternalInput").ap()
    sins = nc.dram_tensor("sins", (128, T), BF16, kind="ExternalInput").ap()
    out = nc.dram_tensor("out", (T, 512), F32, kind="ExternalOutput").ap()
    with tile.TileContext(nc) as tc:
        _emit_body(tc, (xT, wqT, wkT, wvT, cosr, sins, out), T)
    nc.compile()
    return nc


# ---------------- host side ----------------

def _qperm(j):
    rows = []
    for m in range(4):
        for r in range(128):
            h = m if r < 64 else m + 4
            d = 2 * (r % 32) + (1 if (r % 64) >= 32 else 0)
            rows.append((8 * j + h) * 64 + d)
    return np.array(rows)


def _kperm(j):
    rows = []
    for kv in range(2):
        for r in range(64):
            d = 2 * (r % 32) + (1 if r >= 32 else 0)
            rows.append((2 * j + kv) * 64 + d)
    return np.array(rows)


def make_core_inputs(x, Wq, Wk, Wv, cos, sin):
    """Per-core input dicts (host prep). x: [B,T,D]."""
    bf = ml_dtypes.bfloat16
    B, T, _ = x.shape
    xTb = [np.ascontiguousarray(x[b].T).astype(bf) for b in range(B)]
    cosT = np.ascontiguousarray(cos.T.astype(np.float32))  # [32, T]
    sinT = np.ascontiguousarray(sin.T.astype(np.float32))
    cosr = np.tile(cosT, (4, 1)).astype(bf)
    sgn = np.repeat(np.array([-1.0, 1.0, -1.0, 1.0], np.float32), 32)
    sins = (np.tile(sinT, (4, 1)) * sgn[:, None]).astype(bf)
    maps = []
    for c in range(8):
        b, j = c // 4, c % 4
        maps.append({
            "xT": xTb[b],
            "wqT": np.ascontiguousarray(Wq[_qperm(j)].T).astype(bf),
            "wkT": np.ascontiguousarray(Wk[_kperm(j)].T).astype(bf),
            "wvT": np.ascontiguousarray(Wv[128 * j:128 * (j + 1)].T).astype(bf),
            "cosr": cosr,
            "sins": sins,
        })
    return maps


_CACHE = {}


def _get_program():
    if "nc" not in _CACHE:
        _CACHE["nc"] = build_program(T=2048, num_devices=8)
    return _CACHE["nc"]


def run_on_hw(in_maps, trace=False):
    nc = _get_program()
    return run_bass_kernel_spmd(nc, in_maps, list(range(8)), trace=trace)


def kernel(x, Wq, Wk, Wv, cos, sin):
    x = np.asarray(x, np.float32)
    Wq = np.asarray(Wq, np.float32)
    Wk = np.asarray(Wk, np.float32)
    Wv = np.asarray(Wv, np.float32)
    cos = np.asarray(cos, np.float32)
    sin = np.asarray(sin, np.float32)
    maps = make_core_inputs(x, Wq, Wk, Wv, cos, sin)
    res = run_on_hw(maps, trace=False)
    B, T = x.shape[0], x.shape[1]
    out = np.empty((B, T, 2048), np.float32)
    for c in range(8):
        b, j = c // 4, c % 4
        out[b, :, 512 * j:512 * (j + 1)] = res.results[c]["out"]
    return out
